# revision 6
# baseline (speedup 1.0000x reference)
# LoftQ fused kernel for Trainium2 (Bass/Tile), 8-core data-parallel.
#
# reference:
#   W_q = (W_int - zero_point) * scale                  [out=4096, in=4096]
#   W   = W_q + (lora_B @ lora_A) * RANK**-0.5
#   y   = einsum('bsd,od->bso', x, W)                   x: [4, 2048, 4096]
#
# Strategy:
#   - Data-parallel: 8192 tokens sharded 1024/core; W replicated.
#   - y = x @ W_q.T + (x @ A.T) @ (scaling * B.T)
#     The low-rank term never materializes into W: we compute
#     u^T = A_T^T-contractions on the PE (K=128 matmuls, output directly
#     transposed), then append one K=16 tail matmul per output tile.
#   - W_int is fed to the device as packed int8 (values 0..15); dequant
#     (w*scale - zp*scale) runs on the Scalar engine as a single
#     ACTIVATE(Copy, scale, bias) per chunk, producing bf16.
#   - Main GEMM in bf16 (fp32 PSUM accumulation): per (o-chunk, t-tile),
#     32 K-tile matmuls [128x128]@[128x512] + 1 K=16 tail matmul.
#
# Host-side work is limited to sharding/layout packing (transpose + dtype
# packing); all FLOPs (dequant affine, both matmuls) run on device.

import numpy as np
import ml_dtypes

import concourse.bass as bass
import concourse.mybir as mybir
import concourse.tile as tile
from concourse import bacc
from concourse.bass import ts
from concourse.bass_utils import run_bass_kernel_spmd

P = 128
N_CORES = 8
RANK = 16
SCALING = RANK ** (-0.5)
BF16 = mybir.dt.bfloat16
F32 = mybir.dt.float32
I8 = mybir.dt.int8


def build_program(nc, T, D, O, R, scale, bias, OC=512, u_group=4):
    """Emit the per-core program.

    T: tokens per core, D: in_features, O: out_features, R: lora rank.
    scale/bias: dequant immediates (w*scale + bias), bias = -zp*scale.
    Inputs (per core):
      xtp  bf16 [P, D/P, T]      x-shard, transposed+partition-packed
      w8p  int8 [O/OC, P, D/P, OC]  W_int^T, chunk-packed (replicated)
      atp  bf16 [P, D/P, R]      lora_A^T packed (replicated)
      bts  bf16 [R, O]           lora_B^T * scaling (replicated)
    Output: y f32 [T, O]
    """
    DT, TT, NOC = D // P, T // P, O // OC
    xt = nc.dram_tensor("xtp", [P, DT, T], BF16, kind="ExternalInput")
    w8 = nc.dram_tensor("w8p", [NOC, P, DT, OC], I8, kind="ExternalInput")
    at = nc.dram_tensor("atp", [P, DT, R], BF16, kind="ExternalInput")
    bts = nc.dram_tensor("bts", [R, O], BF16, kind="ExternalInput")
    y = nc.dram_tensor("y", [T, O], F32, kind="ExternalOutput")
    y_ap = y.ap().rearrange("(tt p) o -> p tt o", p=P)

    COPY = mybir.ActivationFunctionType.Copy

    with tile.TileContext(nc) as tc:
        with (
            tc.tile_pool(name="const", bufs=1) as cpool,
            tc.tile_pool(name="w8pool", bufs=6) as w8pool,
            tc.tile_pool(name="wtpool", bufs=6) as wtpool,
            tc.tile_pool(name="outpool", bufs=4) as outpool,
            tc.tile_pool(name="psum", bufs=4, space="PSUM") as psum,
            tc.tile_pool(name="psum_u", bufs=u_group, space="PSUM") as psum_u,
        ):
            at_sb = cpool.tile([P, DT, R], BF16)
            nc.sync.dma_start(at_sb[:], at.ap())
            bts_sb = cpool.tile([R, O], BF16)
            nc.sync.dma_start(bts_sb[:], bts.ap())
            xt_sb = cpool.tile([P, DT, T], BF16)
            for dt in range(DT):
                nc.sync.dma_start(xt_sb[:, dt], xt.ap()[:, dt])

            # W chunks arrive/dequant in quarter-tiles so the PE can start
            # a chunk's matmuls after 1/4 of it is ready. Dequant runs on
            # the Vector engine (ACT Copy is ~5x slower per element).
            NQ = 4
            DQ = DT // NQ

            def load_dequant_chunk(oc):
                wqs = []
                for q in range(NQ):
                    w8_sb = w8pool.tile([P, DQ, OC], I8, tag="w8", name=f"w8_{oc}_{q}")
                    nc.sync.dma_start(w8_sb[:], w8.ap()[oc, :, q * DQ : (q + 1) * DQ])
                    wt_sb = wtpool.tile(
                        [P, DQ, OC], BF16, tag="wt", name=f"wt_{oc}_{q}"
                    )
                    nc.vector.tensor_scalar(
                        wt_sb[:],
                        w8_sb[:],
                        scale,
                        bias,
                        mybir.AluOpType.mult,
                        mybir.AluOpType.add,
                    )
                    wqs.append(wt_sb)
                return wqs

            def evict(ps, tt, oc):
                ob = outpool.tile([P, OC], F32, tag="ob", name=f"ob_{oc}_{tt}")
                nc.vector.tensor_copy(ob[:], ps[:])
                nc.sync.dma_start(y_ap[:, tt, ts(oc, OC)], ob[:])

            def tail_mm(ps, tt, oc):
                # K=16 low-rank tail: + u^T[:,t128]^T @ (scaling*B^T)[:, oc]
                nc.tensor.matmul(
                    ps[:],
                    lhsT=ut_sb[:, ts(tt, P)],
                    rhs=bts_sb[:, ts(oc, OC)],
                    start=False,
                    stop=True,
                )

            # u^T = (x @ A^T)^T computed directly transposed, interleaved
            # dt-major with chunk 0's main matmuls so the PE stays busy
            # while the x-shard streams in. Two passes of 4 t-tiles
            # (4 u-psums + 4 main-psums = 8 PSUM banks).
            ut_sb = cpool.tile([R, T], BF16)
            wq0 = load_dequant_chunk(0)
            for g0 in range(0, TT, u_group):
                tts = list(range(g0, min(g0 + u_group, TT)))
                pus = {
                    t: psum_u.tile([R, P], F32, tag="pu", name=f"pu_{t}") for t in tts
                }
                pss = {
                    t: psum.tile([P, OC], F32, tag="ps", name=f"ps0_{t}") for t in tts
                }
                for dt in range(DT):
                    for t in tts:
                        nc.tensor.matmul(
                            pus[t][:],
                            lhsT=at_sb[:, dt],
                            rhs=xt_sb[:, dt, ts(t, P)],
                            start=(dt == 0),
                            stop=(dt == DT - 1),
                        )
                    for t in tts:
                        nc.tensor.matmul(
                            pss[t][:],
                            lhsT=xt_sb[:, dt, ts(t, P)],
                            rhs=wq0[dt // DQ][:, dt % DQ],
                            start=(dt == 0),
                            stop=False,
                        )
                for t in tts:
                    nc.scalar.activation(ut_sb[:, ts(t, P)], pus[t][:], COPY)
                for t in tts:
                    tail_mm(pss[t], t, 0)
                    evict(pss[t], t, 0)

            for oc in range(1, NOC):
                wqs = load_dequant_chunk(oc)
                for tt in range(TT):
                    ps = psum.tile([P, OC], F32, tag="ps", name=f"ps_{oc}_{tt}")
                    for dt in range(DT):
                        nc.tensor.matmul(
                            ps[:],
                            lhsT=xt_sb[:, dt, ts(tt, P)],
                            rhs=wqs[dt // DQ][:, dt % DQ],
                            start=(dt == 0),
                            stop=False,
                        )
                    tail_mm(ps, tt, oc)
                    evict(ps, tt, oc)
    return nc


def _pack_inputs(x, W_int, lora_A, lora_B):
    """Host-side shard + layout packing. Returns per-core input maps."""
    BS, S, D = x.shape
    O = W_int.shape[0]
    Tfull = BS * S
    T = Tfull // N_CORES
    DT = D // P
    OC = 512
    NOC = O // OC

    xb = np.asarray(x, dtype=np.float32).reshape(Tfull, D).astype(ml_dtypes.bfloat16)
    # [oc, p, dt, j] <- W_int^T[d=dt*P+p, o=oc*OC+j]
    w8p = np.ascontiguousarray(
        np.asarray(W_int, dtype=np.int32)
        .T.reshape(DT, P, NOC, OC)
        .transpose(2, 1, 0, 3)
        .astype(np.int8)
    )
    atp = np.ascontiguousarray(
        np.asarray(lora_A, dtype=np.float32)
        .T.reshape(DT, P, RANK)
        .transpose(1, 0, 2)
        .astype(ml_dtypes.bfloat16)
    )
    bts = np.ascontiguousarray(
        (np.asarray(lora_B, dtype=np.float32).T * SCALING).astype(ml_dtypes.bfloat16)
    )
    in_maps = []
    for c in range(N_CORES):
        xs = xb[c * T : (c + 1) * T]  # [T, D] bf16
        xtp = np.ascontiguousarray(xs.T.reshape(DT, P, T).transpose(1, 0, 2))
        in_maps.append({"xtp": xtp, "w8p": w8p, "atp": atp, "bts": bts})
    return in_maps, T, D, O


def _install_ntff_shim():
    """Provide antenv.axon_hooks (absent in this image) so that
    run_bass_kernel_spmd(trace=True) can capture NTFF profiles via the
    axon .so — mirrors trn_agent_boot.trn_boot's degraded-silently path.
    Only used for our own measurement runs (_trace=True)."""
    import sys as _sys
    import types as _types

    if "antenv.axon_hooks" in _sys.modules:
        return
    try:
        from trn_agent_boot.trn_boot import _ntff_profile_via_ctypes
    except ImportError:
        _sys.path.insert(0, "/root/.axon_site")
        from trn_agent_boot.trn_boot import _ntff_profile_via_ctypes

    hook = _ntff_profile_via_ctypes("/opt/axon/libaxon_pjrt.so")
    mod = _types.ModuleType("antenv.axon_hooks")
    mod._hook = hook
    mod.get_axon_ntff_profile_hook = lambda: mod._hook
    mod.set_axon_ntff_profile_hook = lambda h: setattr(mod, "_hook", h)
    _sys.modules["antenv.axon_hooks"] = mod
    import antenv as _antenv

    _antenv.axon_hooks = mod


def kernel(x, W_int, lora_A, lora_B, scale, zero_point, _trace=False, _tmpdir=None):
    if _trace:
        _install_ntff_shim()
    x = np.asarray(x)
    BS, S, D = x.shape
    s = float(np.asarray(scale))
    zp = float(np.asarray(zero_point))
    in_maps, T, D, O = _pack_inputs(x, W_int, lora_A, lora_B)

    nc = bacc.Bacc(
        "TRN2",
        target_bir_lowering=False,
        debug=False,
        num_devices=N_CORES,
    )
    build_program(nc, T, D, O, RANK, scale=s, bias=-zp * s)
    nc.compile()

    res = run_bass_kernel_spmd(
        nc,
        in_maps,
        core_ids=list(range(N_CORES)),
        trace=_trace,
        tmpdir=_tmpdir,
        trace_cores=list(range(N_CORES)) if _trace else None,
    )
    y = np.concatenate([r["y"] for r in res.results], axis=0).reshape(BS, S, O)
    if _trace:
        kernel.last_results = res
    return y


if __name__ == "__main__":
    # smoke: build-only for full shapes
    nc = bacc.Bacc("TRN2", target_bir_lowering=False, debug=False, num_devices=8)
    build_program(nc, 1024, 4096, 4096, 16, scale=0.01, bias=-0.075)
    nc.compile()
    print("build ok; instructions:", sum(len(b.instructions) for b in nc.main_func.blocks))


# revision 7
# speedup vs baseline: 1.0092x; 1.0092x over previous
# LoftQ fused kernel for Trainium2 (Bass/Tile), 8-core data-parallel.
#
# reference:
#   W_q = (W_int - zero_point) * scale                  [out=4096, in=4096]
#   W   = W_q + (lora_B @ lora_A) * RANK**-0.5
#   y   = einsum('bsd,od->bso', x, W)                   x: [4, 2048, 4096]
#
# Strategy:
#   - Data-parallel: 8192 tokens sharded 1024/core; W replicated.
#   - y = x @ W_q.T + (x @ A.T) @ (scaling * B.T)
#     The low-rank term never materializes into W: we compute
#     u^T = A_T^T-contractions on the PE (K=128 matmuls, output directly
#     transposed), then append one K=16 tail matmul per output tile.
#   - W_int is fed to the device as packed int8 (values 0..15); dequant
#     (w*scale - zp*scale) runs on the Scalar engine as a single
#     ACTIVATE(Copy, scale, bias) per chunk, producing bf16.
#   - Main GEMM in bf16 (fp32 PSUM accumulation): per (o-chunk, t-tile),
#     32 K-tile matmuls [128x128]@[128x512] + 1 K=16 tail matmul.
#
# Host-side work is limited to sharding/layout packing (transpose + dtype
# packing); all FLOPs (dequant affine, both matmuls) run on device.

import numpy as np
import ml_dtypes

import concourse.bass as bass
import concourse.mybir as mybir
import concourse.tile as tile
from concourse import bacc
from concourse.bass import ts
from concourse.bass_utils import run_bass_kernel_spmd

P = 128
N_CORES = 8
RANK = 16
SCALING = RANK ** (-0.5)
BF16 = mybir.dt.bfloat16
F32 = mybir.dt.float32
I8 = mybir.dt.int8


def build_program(nc, T, D, O, R, scale, bias, OC=512, u_group=4):
    """Emit the per-core program.

    T: tokens per core, D: in_features, O: out_features, R: lora rank.
    scale/bias: dequant immediates (w*scale + bias), bias = -zp*scale.
    Inputs (per core):
      xtp  bf16 [P, D/P, T]      x-shard, transposed+partition-packed
      w8p  int8 [O/OC, P, D/P, OC]  W_int^T, chunk-packed (replicated)
      atp  bf16 [P, D/P, R]      lora_A^T packed (replicated)
      bts  bf16 [R, O]           lora_B^T * scaling (replicated)
    Output: y f32 [T, O]
    """
    DT, TT, NOC = D // P, T // P, O // OC
    xt = nc.dram_tensor("xtp", [P, DT, T], BF16, kind="ExternalInput")
    w8 = nc.dram_tensor("w8p", [NOC, P, DT, OC], I8, kind="ExternalInput")
    at = nc.dram_tensor("atp", [P, DT, R], BF16, kind="ExternalInput")
    bts = nc.dram_tensor("bts", [R, O], BF16, kind="ExternalInput")
    y = nc.dram_tensor("y", [T, O], F32, kind="ExternalOutput")
    y_ap = y.ap().rearrange("(tt p) o -> p tt o", p=P)

    COPY = mybir.ActivationFunctionType.Copy

    with tile.TileContext(nc) as tc:
        with (
            tc.tile_pool(name="const", bufs=1) as cpool,
            tc.tile_pool(name="w8pool", bufs=6) as w8pool,
            tc.tile_pool(name="wtpool", bufs=6) as wtpool,
            tc.tile_pool(name="outpool", bufs=4) as outpool,
            tc.tile_pool(name="psum", bufs=4, space="PSUM") as psum,
            tc.tile_pool(name="psum_u", bufs=u_group, space="PSUM") as psum_u,
        ):
            at_sb = cpool.tile([P, DT, R], BF16)
            nc.sync.dma_start(at_sb[:], at.ap())
            bts_sb = cpool.tile([R, O], BF16)
            nc.sync.dma_start(bts_sb[:], bts.ap())
            xt_sb = cpool.tile([P, DT, T], BF16)
            for dt in range(DT):
                nc.sync.dma_start(xt_sb[:, dt], xt.ap()[:, dt])

            # W chunks arrive/dequant in quarter-tiles so the PE can start
            # a chunk's matmuls after 1/4 of it is ready. Dequant runs on
            # the Vector engine (ACT Copy is ~5x slower per element).
            NQ = 4
            DQ = DT // NQ

            def load_dequant_chunk(oc):
                wqs = []
                for q in range(NQ):
                    w8_sb = w8pool.tile([P, DQ, OC], I8, tag="w8", name=f"w8_{oc}_{q}")
                    nc.sync.dma_start(w8_sb[:], w8.ap()[oc, :, q * DQ : (q + 1) * DQ])
                    wt_sb = wtpool.tile(
                        [P, DQ, OC], BF16, tag="wt", name=f"wt_{oc}_{q}"
                    )
                    nc.vector.tensor_scalar(
                        wt_sb[:],
                        w8_sb[:],
                        scale,
                        bias,
                        mybir.AluOpType.mult,
                        mybir.AluOpType.add,
                    )
                    wqs.append(wt_sb)
                return wqs

            def evict(ps, tt, oc):
                ob = outpool.tile([P, OC], F32, tag="ob", name=f"ob_{oc}_{tt}")
                nc.vector.tensor_copy(ob[:], ps[:])
                nc.sync.dma_start(y_ap[:, tt, ts(oc, OC)], ob[:])

            def tail_mm(ps, tt, oc):
                # K=16 low-rank tail: + u^T[:,t128]^T @ (scaling*B^T)[:, oc]
                nc.tensor.matmul(
                    ps[:],
                    lhsT=ut_sb[:, ts(tt, P)],
                    rhs=bts_sb[:, ts(oc, OC)],
                    start=False,
                    stop=True,
                )

            # u^T = (x @ A^T)^T, computed directly transposed:
            # psum[r, t128] += at_sb[:, dt]^T @ xt_sb[:, dt, t128]
            ut_sb = cpool.tile([R, T], BF16)
            for g0 in range(0, TT, u_group):
                tts = list(range(g0, min(g0 + u_group, TT)))
                pus = {
                    t: psum_u.tile([R, P], F32, tag="pu", name=f"pu_{t}") for t in tts
                }
                for dt in range(DT):
                    for t in tts:
                        nc.tensor.matmul(
                            pus[t][:],
                            lhsT=at_sb[:, dt],
                            rhs=xt_sb[:, dt, ts(t, P)],
                            start=(dt == 0),
                            stop=(dt == DT - 1),
                        )
                for t in tts:
                    nc.scalar.activation(ut_sb[:, ts(t, P)], pus[t][:], COPY)

            for oc in range(NOC):
                wqs = load_dequant_chunk(oc)
                for tt in range(TT):
                    ps = psum.tile([P, OC], F32, tag="ps", name=f"ps_{oc}_{tt}")
                    for dt in range(DT):
                        nc.tensor.matmul(
                            ps[:],
                            lhsT=xt_sb[:, dt, ts(tt, P)],
                            rhs=wqs[dt // DQ][:, dt % DQ],
                            start=(dt == 0),
                            stop=False,
                        )
                    tail_mm(ps, tt, oc)
                    evict(ps, tt, oc)
    return nc


def _pack_inputs(x, W_int, lora_A, lora_B):
    """Host-side shard + layout packing. Returns per-core input maps."""
    BS, S, D = x.shape
    O = W_int.shape[0]
    Tfull = BS * S
    T = Tfull // N_CORES
    DT = D // P
    OC = 512
    NOC = O // OC

    xb = np.asarray(x, dtype=np.float32).reshape(Tfull, D).astype(ml_dtypes.bfloat16)
    # [oc, p, dt, j] <- W_int^T[d=dt*P+p, o=oc*OC+j]
    w8p = np.ascontiguousarray(
        np.asarray(W_int, dtype=np.int32)
        .T.reshape(DT, P, NOC, OC)
        .transpose(2, 1, 0, 3)
        .astype(np.int8)
    )
    atp = np.ascontiguousarray(
        np.asarray(lora_A, dtype=np.float32)
        .T.reshape(DT, P, RANK)
        .transpose(1, 0, 2)
        .astype(ml_dtypes.bfloat16)
    )
    bts = np.ascontiguousarray(
        (np.asarray(lora_B, dtype=np.float32).T * SCALING).astype(ml_dtypes.bfloat16)
    )
    in_maps = []
    for c in range(N_CORES):
        xs = xb[c * T : (c + 1) * T]  # [T, D] bf16
        xtp = np.ascontiguousarray(xs.T.reshape(DT, P, T).transpose(1, 0, 2))
        in_maps.append({"xtp": xtp, "w8p": w8p, "atp": atp, "bts": bts})
    return in_maps, T, D, O


def _install_ntff_shim():
    """Provide antenv.axon_hooks (absent in this image) so that
    run_bass_kernel_spmd(trace=True) can capture NTFF profiles via the
    axon .so — mirrors trn_agent_boot.trn_boot's degraded-silently path.
    Only used for our own measurement runs (_trace=True)."""
    import sys as _sys
    import types as _types

    if "antenv.axon_hooks" in _sys.modules:
        return
    try:
        from trn_agent_boot.trn_boot import _ntff_profile_via_ctypes
    except ImportError:
        _sys.path.insert(0, "/root/.axon_site")
        from trn_agent_boot.trn_boot import _ntff_profile_via_ctypes

    hook = _ntff_profile_via_ctypes("/opt/axon/libaxon_pjrt.so")
    mod = _types.ModuleType("antenv.axon_hooks")
    mod._hook = hook
    mod.get_axon_ntff_profile_hook = lambda: mod._hook
    mod.set_axon_ntff_profile_hook = lambda h: setattr(mod, "_hook", h)
    _sys.modules["antenv.axon_hooks"] = mod
    import antenv as _antenv

    _antenv.axon_hooks = mod


def kernel(x, W_int, lora_A, lora_B, scale, zero_point, _trace=False, _tmpdir=None):
    if _trace:
        _install_ntff_shim()
    x = np.asarray(x)
    BS, S, D = x.shape
    s = float(np.asarray(scale))
    zp = float(np.asarray(zero_point))
    in_maps, T, D, O = _pack_inputs(x, W_int, lora_A, lora_B)

    nc = bacc.Bacc(
        "TRN2",
        target_bir_lowering=False,
        debug=False,
        num_devices=N_CORES,
    )
    build_program(nc, T, D, O, RANK, scale=s, bias=-zp * s)
    nc.compile()

    res = run_bass_kernel_spmd(
        nc,
        in_maps,
        core_ids=list(range(N_CORES)),
        trace=_trace,
        tmpdir=_tmpdir,
        trace_cores=list(range(N_CORES)) if _trace else None,
    )
    y = np.concatenate([r["y"] for r in res.results], axis=0).reshape(BS, S, O)
    if _trace:
        kernel.last_results = res
    return y


if __name__ == "__main__":
    # smoke: build-only for full shapes
    nc = bacc.Bacc("TRN2", target_bir_lowering=False, debug=False, num_devices=8)
    build_program(nc, 1024, 4096, 4096, 16, scale=0.01, bias=-0.075)
    nc.compile()
    print("build ok; instructions:", sum(len(b.instructions) for b in nc.main_func.blocks))
